# Initial kernel scaffold
#
"""MendGraph kernel for 8 Trainium2 NeuronCores.

Strategy (full-IO contract):
  - Host: k = clip(degree, 0, K); exclusive cumsum -> per-new-node source row
    index idx = src*K + j into generated_features (row gather).
  - Shard nodes contiguously across 8 cores (12500 nodes / 62500 gf rows per
    core).  All gather indices of a core fall inside its own gf shard.
  - Device (per core): row-gather its ~59.6K new-feature rows from its gf
    shard via gpsimd indirect DMA (128 rows per call, one row per SBUF
    partition), store to a dense [T_pad, 128] f32 output.
  - Host: assemble mend_features = [x; gathered rows] and
    mend_edge_index = [edge_index; interleaved new edges] (pure index
    arithmetic, no heavy data movement).
"""

import sys

if "/opt/trn_rl_repo" not in sys.path:
    sys.path.insert(0, "/opt/trn_rl_repo")

import numpy as np

K = 5  # predicated_missing_neighbor_num (fixed by the nn.Module)
M = 8  # NeuronCores

_NC_CACHE = {}


def _build_gather_program(rows, f, n_tiles):
    """One SPMD program: out[t*128+p] = gf[idx[p, t]] for all tiles t."""
    from concourse import bass, mybir
    from concourse.tile import TileContext

    nc = bass.Bass()
    gf_in = nc.declare_dram_parameter("gf", [rows, f], mybir.dt.float32, isOutput=False)
    idx_in = nc.declare_dram_parameter(
        "idx", [128, n_tiles], mybir.dt.int32, isOutput=False
    )
    out = nc.declare_dram_parameter(
        "out", [n_tiles * 128, f], mybir.dt.float32, isOutput=True
    )

    with TileContext(nc) as tc:
        with (
            tc.tile_pool(name="idxp", bufs=1) as idxp,
            tc.tile_pool(name="feat", bufs=8) as featp,
        ):
            idx_sb = idxp.tile([128, n_tiles], mybir.dt.int32)
            nc.sync.dma_start(out=idx_sb[:], in_=idx_in[:])
            for t in range(n_tiles):
                ft = featp.tile([128, f], mybir.dt.float32)
                nc.gpsimd.indirect_dma_start(
                    out=ft[:],
                    out_offset=None,
                    in_=gf_in[:],
                    in_offset=bass.IndirectOffsetOnAxis(ap=idx_sb[:, t : t + 1], axis=0),
                )
                nc.sync.dma_start(out=out[t * 128 : (t + 1) * 128, :], in_=ft[:])
    return nc


def _run_device(gf_shards, idx_ts, rows, f, n_tiles, trace=False):
    from concourse import bass_utils

    key = ("v1", rows, f, n_tiles)
    nc = _NC_CACHE.get(key)
    if nc is None:
        nc = _build_gather_program(rows, f, n_tiles)
        _NC_CACHE[key] = nc
    n = len(gf_shards)
    in_maps = [{"gf": gf_shards[c], "idx": idx_ts[c]} for c in range(n)]
    res = bass_utils.run_bass_kernel_spmd(nc, in_maps, list(range(n)), trace=trace)
    return res


def _prep(degree_np):
    """Host-side index computation shared by kernel() and test tooling."""
    n = degree_np.shape[0]
    npc = n // M
    k = np.clip(degree_np.astype(np.int64), 0, K)
    total = int(k.sum())
    # per-core slices
    kc = k.reshape(M, npc)
    t_c = kc.sum(axis=1)  # new rows per core
    # global exclusive cumsum of per-core totals
    core_off = np.concatenate([[0], np.cumsum(t_c)[:-1]])
    return k, kc, t_c, core_off, total, npc


def _local_indices(kc_c, npc):
    """Gather row indices (local to the core's gf shard) in output order."""
    t_c = int(kc_c.sum())
    if t_c == 0:
        return np.zeros(0, dtype=np.int32)
    base = np.repeat(np.arange(npc, dtype=np.int64) * K, kc_c)
    excl = np.cumsum(kc_c) - kc_c
    j = np.arange(t_c, dtype=np.int64) - np.repeat(excl, kc_c)
    return (base + j).astype(np.int32)


def kernel(x, edge_index, degree, generated_features):
    x = np.ascontiguousarray(np.asarray(x, dtype=np.float32))
    edge_index = np.asarray(edge_index)
    degree_np = np.asarray(degree)
    gf = np.ascontiguousarray(np.asarray(generated_features, dtype=np.float32))

    n, f = x.shape
    k, kc, t_c, core_off, total, npc = _prep(degree_np)
    if total == 0:
        return x, edge_index

    rows = npc * K  # gf rows per core shard
    t_pad = ((int(t_c.max()) + 127) // 128) * 128
    n_tiles = t_pad // 128

    gf_sh = gf.reshape(M, rows, f)
    gf_shards = [np.ascontiguousarray(gf_sh[c]) for c in range(M)]
    idx_ts = []
    for c in range(M):
        idx = _local_indices(kc[c], npc)
        ipad = np.zeros(t_pad, dtype=np.int32)
        ipad[: idx.shape[0]] = idx
        # device consumes [128, n_tiles]: tile t gathers rows t*128..t*128+127
        idx_ts.append(np.ascontiguousarray(ipad.reshape(n_tiles, 128).T))

    res = _run_device(gf_shards, idx_ts, rows, f, n_tiles, trace=False)

    # ---- host assembly ----
    feats = np.empty((n + total, f), dtype=np.float32)
    feats[:n] = x
    for c in range(M):
        tc_ = int(t_c[c])
        feats[n + core_off[c] : n + core_off[c] + tc_] = res.results[c]["out"][:tc_]

    src = np.repeat(np.arange(n, dtype=np.int64), k)
    new_ids = n + np.arange(total, dtype=np.int64)
    dt = edge_index.dtype
    new_edges = np.empty((2, 2 * total), dtype=dt)
    new_edges[0, 0::2] = src
    new_edges[0, 1::2] = new_ids
    new_edges[1, 0::2] = new_ids
    new_edges[1, 1::2] = src
    e = edge_index.shape[1]
    mend_edge_index = np.empty((2, e + 2 * total), dtype=dt)
    mend_edge_index[:, :e] = edge_index
    mend_edge_index[:, e:] = new_edges
    return feats, mend_edge_index


# revision 3
# speedup vs baseline: 29138.2189x; 29138.2189x over previous
"""MendGraph kernel for 8 Trainium2 NeuronCores.

Strategy (full-IO contract):
  - Host: k = clip(degree, 0, K); exclusive cumsum -> per-new-node source row
    index idx = src*K + j into generated_features (row gather).
  - Shard nodes contiguously across 8 cores (12500 nodes / 62500 gf rows per
    core).  All gather indices of a core fall inside its own gf shard.
  - Device (per core): row-gather its ~59.6K new-feature rows from its gf
    shard via gpsimd indirect DMA (128 rows per call, one row per SBUF
    partition), store to a dense [T_pad, 128] f32 output.
  - Host: assemble mend_features = [x; gathered rows] and
    mend_edge_index = [edge_index; interleaved new edges] (pure index
    arithmetic, no heavy data movement).
"""

import sys

if "/opt/trn_rl_repo" not in sys.path:
    sys.path.insert(0, "/opt/trn_rl_repo")

import numpy as np

K = 5  # predicated_missing_neighbor_num (fixed by the nn.Module)
M = 8  # NeuronCores

_NC_CACHE = {}


def _build_gather_program(rows, f, n_tiles, repeat=1):
    """One SPMD program: out[t*128+p] = gf[idx[p, t]] for all tiles t."""
    from concourse import bass, bacc, mybir
    from concourse.tile import TileContext

    nc = bacc.Bacc("TRN2", target_bir_lowering=False, debug=False, num_devices=M)
    gf_in = nc.dram_tensor("gf", [rows, f], mybir.dt.float32, kind="ExternalInput").ap()
    idx_in = nc.dram_tensor(
        "idx", [128, n_tiles], mybir.dt.int32, kind="ExternalInput"
    ).ap()
    out = nc.dram_tensor(
        "out", [n_tiles * 128, f], mybir.dt.float32, kind="ExternalOutput"
    ).ap()

    with TileContext(nc) as tc:
        with (
            tc.tile_pool(name="idxp", bufs=1) as idxp,
            tc.tile_pool(name="feat", bufs=8) as featp,
        ):
            idx_sb = idxp.tile([128, n_tiles], mybir.dt.int32)
            nc.sync.dma_start(out=idx_sb[:], in_=idx_in[:])
            for _ in range(repeat):
                for t in range(n_tiles):
                    ft = featp.tile([128, f], mybir.dt.float32)
                    nc.gpsimd.indirect_dma_start(
                        out=ft[:],
                        out_offset=None,
                        in_=gf_in[:],
                        in_offset=bass.IndirectOffsetOnAxis(
                            ap=idx_sb[:, t : t + 1], axis=0
                        ),
                    )
                    nc.sync.dma_start(out=out[t * 128 : (t + 1) * 128, :], in_=ft[:])
    nc.compile()
    return nc


def _run_device(gf_shards, idx_ts, rows, f, n_tiles, trace=False):
    from concourse import bass_utils

    key = ("v1", rows, f, n_tiles)
    nc = _NC_CACHE.get(key)
    if nc is None:
        nc = _build_gather_program(rows, f, n_tiles)
        _NC_CACHE[key] = nc
    n = len(gf_shards)
    in_maps = [{"gf": gf_shards[c], "idx": idx_ts[c]} for c in range(n)]
    res = bass_utils.run_bass_kernel_spmd(nc, in_maps, list(range(n)), trace=trace)
    return res


def _prep(degree_np):
    """Host-side index computation shared by kernel() and test tooling."""
    n = degree_np.shape[0]
    npc = n // M
    k = np.clip(degree_np.astype(np.int64), 0, K)
    total = int(k.sum())
    # per-core slices
    kc = k.reshape(M, npc)
    t_c = kc.sum(axis=1)  # new rows per core
    # global exclusive cumsum of per-core totals
    core_off = np.concatenate([[0], np.cumsum(t_c)[:-1]])
    return k, kc, t_c, core_off, total, npc


def _local_indices(kc_c, npc):
    """Gather row indices (local to the core's gf shard) in output order."""
    t_c = int(kc_c.sum())
    if t_c == 0:
        return np.zeros(0, dtype=np.int32)
    base = np.repeat(np.arange(npc, dtype=np.int64) * K, kc_c)
    excl = np.cumsum(kc_c) - kc_c
    j = np.arange(t_c, dtype=np.int64) - np.repeat(excl, kc_c)
    return (base + j).astype(np.int32)


def kernel(x, edge_index, degree, generated_features):
    x = np.ascontiguousarray(np.asarray(x, dtype=np.float32))
    edge_index = np.asarray(edge_index)
    degree_np = np.asarray(degree)
    gf = np.ascontiguousarray(np.asarray(generated_features, dtype=np.float32))

    n, f = x.shape
    k, kc, t_c, core_off, total, npc = _prep(degree_np)
    if total == 0:
        return x, edge_index

    rows = npc * K  # gf rows per core shard
    t_pad = ((int(t_c.max()) + 127) // 128) * 128
    n_tiles = t_pad // 128

    gf_sh = gf.reshape(M, rows, f)
    gf_shards = [np.ascontiguousarray(gf_sh[c]) for c in range(M)]
    idx_ts = []
    for c in range(M):
        idx = _local_indices(kc[c], npc)
        ipad = np.zeros(t_pad, dtype=np.int32)
        ipad[: idx.shape[0]] = idx
        # device consumes [128, n_tiles]: tile t gathers rows t*128..t*128+127
        idx_ts.append(np.ascontiguousarray(ipad.reshape(n_tiles, 128).T))

    res = _run_device(gf_shards, idx_ts, rows, f, n_tiles, trace=False)

    # ---- host assembly ----
    feats = np.empty((n + total, f), dtype=np.float32)
    feats[:n] = x
    for c in range(M):
        tc_ = int(t_c[c])
        feats[n + core_off[c] : n + core_off[c] + tc_] = res.results[c]["out"][:tc_]

    src = np.repeat(np.arange(n, dtype=np.int64), k)
    new_ids = n + np.arange(total, dtype=np.int64)
    dt = edge_index.dtype
    new_edges = np.empty((2, 2 * total), dtype=dt)
    new_edges[0, 0::2] = src
    new_edges[0, 1::2] = new_ids
    new_edges[1, 0::2] = new_ids
    new_edges[1, 1::2] = src
    e = edge_index.shape[1]
    mend_edge_index = np.empty((2, e + 2 * total), dtype=dt)
    mend_edge_index[:, :e] = edge_index
    mend_edge_index[:, e:] = new_edges
    return feats, mend_edge_index
